# revision 2
# baseline (speedup 1.0000x reference)
"""DLRM pairwise-interaction kernel for Trainium2 (8 NeuronCores), v3.

Computes, for each batch b: Z_b = X_b @ X_b^T (X_b is [64, 256]) and emits the
strict lower triangle row-major -> [B, 2016] fp32.

The triangle compaction is descriptor-bound: one DMA descriptor per
(batch, tri-row), 64512 per core. Measured TRN2 rule: descriptors rotate
SDMA engines per SBUF-side partition entry (source side for SBUF-sourced
DMAs, dest side for DRAM->SBUF). An SBUF-sourced gather from Z's
[(g,i) part, (pair,j) col] layout touches only 2 partitions per tri-row, so
its descriptors pile onto engines 0/1 (the baseline bottleneck, rings ~96%
busy). Fix: round-trip Z through an HBM scratch so the gather runs
DRAM->SBUF, whose engine assignment follows the 128-partition destination:

  1. input: 2MB fp32 loads (2048 desc, spread), cast fp16 on DVE/ACT.
  2. PE-transpose pairs, Z = XT^T @ XT in fp32 PSUM (two batches column-tiled
     per matmul), DVE evacuates each [128, 512] PSUM bank to a small slab.
  3. slab -> HBM scratch zh [128 rows = (g, n1), 32768 cols = (pair, n2)]:
     dense 2KB descriptors, spread, hidden under the input stream.
  4. tail: per (g, tri-row i): gather zh[g*64+i, pair-cols 0:i] ->
     out_g[128 part = pair-high, 4 x 2016]; 126 DMAs, 512 descriptors each,
     engine rotation follows out_g's 128 partitions -> all 16 engines.
  5. two dense stores out_g -> y (8KB descriptors, spread).

Batch mapping: pair k = batches (2k, 2k+1), k = p*4 + kl in the gather/store
(p = out partition, kl = column group), so y row = 8p + 2kl + g.
"""
import sys
import numpy as np

sys.path.insert(0, "/opt/trn_rl_repo")

import concourse.bass as bass
import concourse.mybir as mybir
import concourse.tile as tile
from concourse.vector_clock import ScopedClock

F32 = mybir.dt.float32
F16 = mybir.dt.float16

B, N, D, TRI = 8192, 64, 256, 2016
NCORES = 8
NB = B // NCORES           # batches per core
NPAIRS = NB // 2           # 512
CHUNK_PAIRS = 16           # pairs per input DMA (2 MB fp32 read)

# ---------------------------------------------------------------------------
# Workaround for walrus builds that only accept ONE sync-wait per instruction:
# hoist all-but-one wait onto NoOp instructions committed just before, on the
# same engine (same-engine program order preserves semantics).
# ---------------------------------------------------------------------------
_orig_commit = tile.TileContext._commit_instruction


def _split_waits(self, inst):
    si = getattr(inst, "sync_info", None)
    if si is None or not si.on_wait or len(si.on_wait) <= 1:
        return
    if inst.engine == mybir.EngineType.Unassigned:
        return
    waits = list(si.on_wait)
    inst.sync_info = mybir.SyncInfo(on_wait=[waits[-1]], on_update=list(si.on_update))
    for w in waits[:-1]:
        nop = mybir.InstNoOp(name=f"{inst.name}-wsplit-{w.id}", ins=[], outs=[])
        nop.engine = inst.engine
        nop.sync_info = mybir.SyncInfo(on_wait=[w], on_update=[])
        _orig_commit(self, nop, lazy_reg_writes=False)


def _commit_instruction_split(self, inst, lazy_reg_writes=True):
    _split_waits(self, inst)
    return _orig_commit(self, inst, lazy_reg_writes=lazy_reg_writes)


def _drain_and_barrier_split(self, tick_clock, wait_clock):
    drain_inst = self.nc.sync.drain()
    wait_clock.add_sem_waits(
        drain_inst.ins, ScopedClock({None: tick_clock.global_clock})
    )
    si = drain_inst.ins.sync_info
    if si is not None and si.on_wait and len(si.on_wait) > 1:
        waits = list(si.on_wait)
        drain_inst.ins.sync_info = mybir.SyncInfo(
            on_wait=[waits[0]], on_update=list(si.on_update)
        )
        for w in waits[1:]:
            nop = self.nc.sync.nop(nofuse=True)
            nop.ins.sync_info = mybir.SyncInfo(on_wait=[w], on_update=[])

    self.nc.all_engine_barrier()
    assert self.sems is not None
    popped = self.nc._tile_sem_poison_stack.pop()
    assert popped is self._sem_poison
    self.nc.clear_and_free_semaphores(list(self.sems.allocated().values()))
    self.nc.all_engine_barrier()


def _install_tile_workarounds():
    tile.TileContext._commit_instruction = _commit_instruction_split
    tile.TileContext._drain_and_barrier = _drain_and_barrier_split


def build_program(nb=NB, chunk_pairs=CHUNK_PAIRS):
    _install_tile_workarounds()
    npairs = nb // 2
    nchunks = npairs // chunk_pairs   # 32

    nc = bass.Bass("TRN2", target_bir_lowering=False, debug=False,
                   num_devices=NCORES)
    x = nc.dram_tensor("x", [nb, N, D], F32, kind="ExternalInput").ap()
    ident = nc.dram_tensor("ident", [128, 128], F16, kind="ExternalInput").ap()
    y = nc.dram_tensor("y", [nb, TRI], F32, kind="ExternalOutput").ap()
    xflat = x.rearrange("b n d -> (b n) d")

    with tile.TileContext(nc) as tc:
        with (
            tc.tile_pool(name="const", bufs=1) as constp,
            tc.tile_pool(name="xf32", bufs=2) as xf32p,
            tc.tile_pool(name="xin", bufs=3) as xinp,
            tc.tile_pool(name="xt", bufs=3) as xtp_sb,
            tc.tile_pool(name="zslab", bufs=3) as zslabp,
            tc.tile_pool(name="og", bufs=2) as ogp,
            tc.tile_pool(name="zh", bufs=1, space="DRAM") as zhp,
            tc.tile_pool(name="xtps", bufs=3, space="PSUM") as xtps,
            tc.tile_pool(name="zps", bufs=3, space="PSUM") as zps,
        ):
            ident_sb = constp.tile([128, 128], F16)
            nc.sync.dma_start(ident_sb[:], ident[:])

            # HBM scratch: rows (g, n1), cols (pair k, n2)
            zh = zhp.tile([128, npairs * 64], F32, tag="zh")

            for c in range(nchunks):
                # ---- load X (fp32, sync HWDGE) + cast on DVE/ACT ----------
                xf = xf32p.tile([128, chunk_pairs * 256], F32, tag="xf32")
                row0 = c * chunk_pairs * 128
                src = xflat[row0:row0 + chunk_pairs * 128, :].rearrange(
                    "(l p) d -> p l d", p=128)
                nc.sync.dma_start(
                    xf[:].rearrange("p (l d) -> p l d", d=256), src)
                xb = xinp.tile([128, chunk_pairs * 256], F16, tag="xin")
                if c % 2 == 0:
                    nc.vector.tensor_copy(xb[:], xf[:])
                else:
                    nc.scalar.copy(xb[:], xf[:])

                # ---- per q8 group (8 pairs): transpose + matmuls + evac ----
                for q8 in range(chunk_pairs // 8):
                    zp = zps.tile([128, 512], F32, tag="zps")
                    for half in range(2):
                        q4 = q8 * 2 + half
                        xtp = xtps.tile([128, 1024], F16, tag="xtps")
                        for pl in range(4):
                            lc = q4 * 4 + pl
                            for cc in range(2):
                                nc.tensor.transpose(
                                    xtp[:, pl * 256 + cc * 128:pl * 256 + (cc + 1) * 128],
                                    xb[:, lc * 256 + cc * 128:lc * 256 + (cc + 1) * 128],
                                    ident_sb[:])
                        xt = xtp_sb.tile([128, 1024], F16, tag="xt")
                        nc.vector.tensor_copy(xt[:], xtp[:])
                        for pl in range(4):
                            slot = half * 4 + pl
                            q0 = pl * 256
                            q1 = pl * 256 + 128
                            zsl = zp[:, slot * 64:(slot + 1) * 64]
                            nc.tensor.matmul(zsl[0:64, :], xt[:, q0:q0 + 64],
                                             xt[:, q0:q0 + 64],
                                             start=True, stop=False,
                                             skip_group_check=True)
                            nc.tensor.matmul(zsl[64:128, :], xt[:, q0 + 64:q0 + 128],
                                             xt[:, q0 + 64:q0 + 128],
                                             start=True, stop=False,
                                             skip_group_check=True)
                            nc.tensor.matmul(zsl[0:64, :], xt[:, q1:q1 + 64],
                                             xt[:, q1:q1 + 64],
                                             start=False, stop=True,
                                             skip_group_check=True)
                            nc.tensor.matmul(zsl[64:128, :], xt[:, q1 + 64:q1 + 128],
                                             xt[:, q1 + 64:q1 + 128],
                                             start=False, stop=True,
                                             skip_group_check=True)
                    # evac PSUM -> slab -> dense dump into HBM scratch
                    zs = zslabp.tile([128, 512], F32, tag="zslab")
                    nc.vector.tensor_copy(zs[:], zp[:])
                    kbase = c * chunk_pairs + q8 * 8
                    eng = nc.scalar if q8 % 2 == 0 else nc.sync
                    eng.dma_start(zh[:, kbase * 64:(kbase + 8) * 64], zs[:])

            # ---- tail: triangle gather DRAM->SBUF, then dense stores ------
            # batch b = 2k+g, pair k = p*4 + kl  (p = out partition)
            zhr = zh[:].rearrange("r (p kl j) -> r p kl j", p=128, j=64)
            outs = []
            for g in range(2):
                og = ogp.tile([128, 4 * TRI], F32, tag=f"og{g}")
                ogr = og[:].rearrange("p (kl t) -> p kl t", kl=4)
                outs.append(og)
                for i in range(1, 64):
                    off = i * (i - 1) // 2
                    eng = nc.scalar if i % 2 == 0 else nc.sync
                    eng.dma_start(ogr[:, :, off:off + i],
                                  zhr[g * 64 + i, :, :, 0:i])
            for g in range(2):
                ydst = y.rearrange("(p kl g) t -> g p kl t", g=2, kl=4)[g]
                osrc = outs[g][:].rearrange("p (kl t) -> p kl t", kl=4)
                eng = nc.scalar if g == 0 else nc.sync
                eng.dma_start(ydst, osrc)
    return nc


_PROGRAM_CACHE = {}


def _get_program():
    if "nc" not in _PROGRAM_CACHE:
        _PROGRAM_CACHE["nc"] = build_program()
    return _PROGRAM_CACHE["nc"]


def kernel(inputs):
    from concourse.bass_utils import run_bass_kernel_spmd

    x = np.asarray(inputs, dtype=np.float32)
    assert x.shape == (B, N, D), x.shape
    nc = _get_program()
    eye = np.eye(128, dtype=np.float16)
    in_maps = [
        {"x": np.ascontiguousarray(x[i * NB:(i + 1) * NB]), "ident": eye}
        for i in range(NCORES)
    ]
    res = run_bass_kernel_spmd(nc, in_maps, list(range(NCORES)))
    out = np.concatenate([res.results[i]["y"] for i in range(NCORES)], axis=0)
    return out.astype(np.float32, copy=False)
